# revision 10
# baseline (speedup 1.0000x reference)
"""GAT layer kernel for 8 Trainium2 NeuronCores (Bass/Tile).

Strategy (dst-tile edge sharding - no softmax all-reduce needed):
  * Nodes padded to 50176 rows = 392 tiles of 128; core c owns dst tiles
    [49c, 49c+49) and computes z = h @ W for the same node range, with the
    attention partial dots s1 = z@w1, s2 = z@w2 folded into the matmul
    (v1 = fc_w.T @ w1).  Rows are stored as a 256-col fp16 table
    [z(128) | 1 | s1 | s2 | pad] and AllGathered so every core can gather
    any src row.
  * Per edge the core gathers the 512B table row of the edge's src via the
    MoE dma_gather primitive (int16 indices; the table is split at row
    32768 into two gather calls per batch).
  * Per dst tile a one-hot matrix O[e, dst_local], scaled by the per-edge
    softmax numerator exp(e), is the matmul lhsT: PSUM accumulates
    [S | denom] = O^T @ [z_rows | 1] in one shot; h_out tile = S / denom.
  * s2[dst] per edge is recovered on-chip from the unscaled one-hot:
    s2pe = reduce_X(O01 * s2row), where s2row is the tile's s2 column
    (local z_slice) replicated across partitions by a rank-1 matmul.
  * exp(e - max) is computed without the max subtraction: logits are
    poisson-pmf-weighted leaky-relu values bounded by ~0.3, so exp is
    numerically safe and the softmax is unchanged.

Host does integer/index preprocessing (edge bucketing, index layout,
casts of index-derived small arrays) and weight folding only; all
data-dependent float math runs on device.
"""

import math
import os
import sys

import numpy as np

for _p in ("/opt/trn_rl_repo", os.path.expanduser("~/.axon_site/_ro/trn_rl_repo")):
    if os.path.isdir(_p) and _p not in sys.path:
        sys.path.insert(0, _p)
        break

import concourse.bass as bass  # noqa: F401
import concourse.bacc as bacc
import concourse.mybir as mybir
import concourse.tile as tile
from concourse.bass_utils import run_bass_kernel_spmd

F32 = mybir.dt.float32
F16 = mybir.dt.float16
I32 = mybir.dt.int32
I16 = mybir.dt.int16

N_NODES = 50000
N_EDGES = 500000
IN_DIM = 256
OUT_DIM = 128
NEG_SLOPE = 0.01

NC = 8
P = 128
TPC = 49                    # real dst tiles per core (392 = 8*49)
RPC = TPC * P               # 6272 real node rows per core
TCORE = 50                  # tile slots per core (last one is padding)
SPC = TCORE * P             # 6400 table rows per core slice (incl. pad)
NPAD = NC * SPC             # 51200 table rows total
TROW = 256                  # fp16 elements per table row (512 B)
C_ONE = 128                 # table col: constant 1.0
C_S1 = 129                  # table col: s1 = z @ w1
C_S2 = 130                  # table col: s2 = z @ w2
LOWSPLIT = 32768            # int16 index split point (table rows)
TPB = 5                     # dst tiles per batch
NB = TCORE // TPB           # 10 batches
DIST_COLS = 3907            # 128*3907 = 500096 >= N_EDGES

_prog_cache = {}


def _table_row(node):
    """node id -> row in the padded/allgathered table."""
    return (node // RPC) * SPC + (node % RPC)


def _build_program(LOWC, HIGHC):
    STAGE = int(os.environ.get("GAT_STAGE", "9"))
    CPT = LOWC + HIGHC
    CPB = TPB * CPT
    NCHUNK = TCORE * CPT
    SEGL = TPB * LOWC * P // 16   # int16 idx cols per batch (low call)
    SEGH = TPB * HIGHC * P // 16

    nc = bacc.Bacc()

    hT = nc.declare_dram_parameter("hT", [IN_DIM, RPC], F32, isOutput=False)
    W = nc.declare_dram_parameter("W", [IN_DIM, TROW], F16, isOutput=False)
    distp = nc.declare_dram_parameter("distp", [P, DIST_COLS], I32, isOutput=False)
    gil = nc.declare_dram_parameter("gil", [P, NB * SEGL], I16, isOutput=False)
    gih = nc.declare_dram_parameter("gih", [P, NB * SEGH], I16, isOutput=False)
    dl = nc.declare_dram_parameter("dl", [P, NCHUNK], F16, isOutput=False)
    bits = nc.declare_dram_parameter("bits", [P, 6 * NCHUNK], F16, isOutput=False)
    Bf = nc.declare_dram_parameter("Bf", [P, NCHUNK], F32, isOutput=False)
    iot = nc.declare_dram_parameter("iot", [P, CPB * P], F16, isOutput=False)
    hout = nc.declare_dram_parameter("hout", [TCORE * P, OUT_DIM], F32, isOutput=True)
    dbg_ex = nc.declare_dram_parameter("dbg_ex", [P, NCHUNK], F32, isOutput=True)
    dbg_s2 = nc.declare_dram_parameter("dbg_s2", [P, NCHUNK], F32, isOutput=True)
    dbg_s1 = nc.declare_dram_parameter("dbg_s1", [P, NCHUNK], F32, isOutput=True)
    dbg_dn = nc.declare_dram_parameter("dbg_dn", [TCORE * P, 1], F32, isOutput=True)

    z_slice = nc.dram_tensor("z_slice", [SPC, TROW], F16)
    z_all = nc.dram_tensor("z_all", [NPAD, TROW], F16, addr_space="Shared")

    with tile.TileContext(nc) as tc:
        with (
            tc.tile_pool(name="persist", bufs=1) as pp,
            tc.tile_pool(name="ppsum", bufs=1, space="PSUM") as pps,
        ):
            ones1 = pp.tile([1, P], F16)          # K=1 replicator lhsT
            nc.vector.memset(ones1[:], 1.0)
            ones1f = pp.tile([1, P], F32)
            nc.vector.memset(ones1f[:], 1.0)
            ones128 = pp.tile([P, 1], F32)
            nc.vector.memset(ones128[:], 1.0)

            # ---------------- mu = mean(dist) ----------------
            dsum = pp.tile([P, 1], F32)
            with tc.tile_pool(name="distp_pool", bufs=1) as dpp:
                dist_i = dpp.tile([P, DIST_COLS], I32)
                nc.sync.dma_start(out=dist_i[:], in_=distp[:])
                dist_f = dpp.tile([P, DIST_COLS], F32)
                nc.vector.tensor_copy(out=dist_f[:], in_=dist_i[:])
                nc.vector.tensor_reduce(
                    out=dsum[:], in_=dist_f[:], axis=mybir.AxisListType.X,
                    op=mybir.AluOpType.add,
                )
            mu1_ps = pps.tile([1, 1], F32, space="PSUM")
            nc.tensor.matmul(out=mu1_ps[:], lhsT=dsum[:], rhs=ones128[:],
                             start=True, stop=True)
            mu1 = pp.tile([1, 1], F32)
            nc.vector.tensor_scalar_mul(out=mu1[:], in0=mu1_ps[:],
                                        scalar1=1.0 / N_EDGES)
            mub_ps = pps.tile([P, 1], F32, space="PSUM")
            nc.tensor.matmul(out=mub_ps[:], lhsT=ones1f[:], rhs=mu1[:],
                             start=True, stop=True)
            mu_b = pp.tile([P, 1], F32)
            nc.vector.tensor_copy(out=mu_b[:], in_=mub_ps[:])
            negmu = pp.tile([P, 1], F32)
            nc.vector.tensor_scalar_mul(out=negmu[:], in0=mu_b[:], scalar1=-1.0)
            expnegmu = pp.tile([P, 1], F32)
            nc.scalar.activation(out=expnegmu[:], in_=negmu[:],
                                 func=mybir.ActivationFunctionType.Exp)
            # (mu/64)^(2^b) ladder, minus one: exact f32 chain
            mpow = pp.tile([P, 6], F32)
            nc.vector.tensor_scalar_mul(out=mpow[:, 0:1], in0=mu_b[:],
                                        scalar1=1.0 / 64.0)
            for b_ in range(1, 6):
                nc.vector.tensor_tensor(out=mpow[:, b_:b_ + 1],
                                        in0=mpow[:, b_ - 1:b_],
                                        in1=mpow[:, b_ - 1:b_],
                                        op=mybir.AluOpType.mult)
            mpm1 = pp.tile([P, 6], F32)
            nc.vector.tensor_scalar_add(out=mpm1[:], in0=mpow[:], scalar1=-1.0)

            # ---------------- phase 1: z table ----------------
            with (
                tc.tile_pool(name="zphase", bufs=1) as zp,
                tc.tile_pool(name="ztile", bufs=3) as ztp,
                tc.tile_pool(name="zpsum", bufs=2, space="PSUM") as zps,
            ):
                w_sb = [zp.tile([P, TROW], F16, tag=f"w{k}", name=f"w{k}")
                        for k in range(IN_DIM // P)]
                h_sb = [zp.tile([P, RPC], F16, tag=f"h{k}", name=f"h{k}")
                        for k in range(IN_DIM // P)]
                for k in range(IN_DIM // P):
                    nc.sync.dma_start(out=w_sb[k][:], in_=W[k * P:(k + 1) * P, :])
                    # f32 -> fp16 cast in flight (SWDGE)
                    nc.gpsimd.dma_start(out=h_sb[k][:],
                                        in_=hT[k * P:(k + 1) * P, :])
                for i in range(TPC):
                    zps_t = zps.tile([P, TROW], F32, space="PSUM")
                    for k in range(IN_DIM // P):
                        nc.tensor.matmul(
                            out=zps_t[:],
                            lhsT=h_sb[k][:, i * P:(i + 1) * P],
                            rhs=w_sb[k][:],
                            start=(k == 0),
                            stop=(k == IN_DIM // P - 1),
                        )
                    zt = ztp.tile([P, TROW], F16)
                    nc.vector.tensor_copy(out=zt[:], in_=zps_t[:])
                    nc.vector.memset(zt[:, C_ONE:C_ONE + 1], 1.0)
                    nc.sync.dma_start(out=z_slice[i * P:(i + 1) * P, :], in_=zt[:])
                # zero the padding tile of the slice
                zz = ztp.tile([P, TROW], F16)
                nc.vector.memset(zz[:], 0.0)
                nc.sync.dma_start(out=z_slice[TPC * P:SPC, :], in_=zz[:])

            # ---------------- allgather ----------------
            nc.gpsimd.collective_compute(
                "AllGather",
                mybir.AluOpType.bypass,
                replica_groups=[list(range(NC))],
                ins=[z_slice[:]],
                outs=[z_all[:]],
            )

            # ---------------- edge-phase inputs ----------------
            gil_sb = pp.tile([P, NB * SEGL], I16)
            nc.sync.dma_start(out=gil_sb[:], in_=gil[:])
            gih_sb = pp.tile([P, NB * SEGH], I16)
            nc.sync.dma_start(out=gih_sb[:], in_=gih[:])
            dl_sb = pp.tile([P, NCHUNK], F16)
            nc.sync.dma_start(out=dl_sb[:], in_=dl[:])
            bits_sb = pp.tile([P, 6 * NCHUNK], F16)
            nc.sync.dma_start(out=bits_sb[:], in_=bits[:])
            Bf_sb = pp.tile([P, NCHUNK], F32)
            nc.sync.dma_start(out=Bf_sb[:], in_=Bf[:])
            iot_sb = pp.tile([P, CPB * P], F16)
            nc.sync.dma_start(out=iot_sb[:], in_=iot[:])

            # ---------------- phase 2: edge batches ----------------
            with (
                tc.tile_pool(name="rows", bufs=2) as rp,
                tc.tile_pool(name="oh", bufs=2) as op_,
                tc.tile_pool(name="small", bufs=2) as sp,
                tc.tile_pool(name="spsum", bufs=3, space="PSUM") as sps,
                tc.tile_pool(name="outp", bufs=3) as outp,
            ):
                for b in range(NB if STAGE >= 2 else 0):
                    rows = rp.tile([P, CPB, TROW], F16, tag="rows")
                    nc.gpsimd.dma_gather(
                        out_ap=rows[:, 0:TPB * LOWC, :],
                        in_ap=z_all[:],
                        idxs_ap=gil_sb[:, b * SEGL:(b + 1) * SEGL],
                        num_idxs=TPB * LOWC * P,
                        num_idxs_reg=TPB * LOWC * P,
                        elem_size=TROW,
                        single_packet=False,
                    )
                    nc.gpsimd.dma_gather(
                        out_ap=rows[:, TPB * LOWC:CPB, :],
                        in_ap=z_all[LOWSPLIT:, :],
                        idxs_ap=gih_sb[:, b * SEGH:(b + 1) * SEGH],
                        num_idxs=TPB * HIGHC * P,
                        num_idxs_reg=TPB * HIGHC * P,
                        elem_size=TROW,
                        single_packet=False,
                    )

                    if STAGE < 3:
                        continue
                    # s2row: per tile its 128 s2 values, replicated across
                    # partitions via a K=1 outer-product matmul.
                    s2row = sp.tile([P, TPB, P], F16, tag="s2row")
                    for t in range(TPB):
                        ti = b * TPB + t
                        s2v = sp.tile([1, P], F16, tag="s2v")
                        nc.sync.dma_start(
                            out=s2v[:],
                            in_=z_slice[ti * P:(ti + 1) * P, C_S2:C_S2 + 1]
                                .rearrange("r one -> one r"),
                        )
                        s2ps = sps.tile([P, P], F32, space="PSUM", tag="s2ps")
                        nc.tensor.matmul(out=s2ps[:], lhsT=ones1[:], rhs=s2v[:],
                                         start=True, stop=True)
                        nc.vector.tensor_copy(out=s2row[:, t, :], in_=s2ps[:])

                    if STAGE < 4:
                        continue
                    # one-hot 0/1: O01[p, c, j] = (iota_j == dl[p, c])
                    o01 = op_.tile([P, CPB, P], F16, tag="o01")
                    nc.vector.tensor_tensor(
                        out=o01[:],
                        in0=iot_sb[:].rearrange("p (c j) -> p c j", j=P),
                        in1=dl_sb[:, b * CPB:(b + 1) * CPB]
                            .unsqueeze(2).broadcast_to([P, CPB, P]),
                        op=mybir.AluOpType.is_equal,
                    )
                    # s2 per edge: reduce_X(O01 * s2row).  The batch chunk
                    # layout is region-major: [5 tiles x LOWC | 5 tiles x
                    # HIGHC], so broadcast s2row per region.
                    tmp = op_.tile([P, CPB, P], F16, tag="scratch", name="tmp")
                    L = TPB * LOWC
                    nc.vector.tensor_tensor(
                        out=tmp[:, 0:L, :].rearrange("p (t c) j -> p t c j", t=TPB),
                        in0=o01[:, 0:L, :].rearrange("p (t c) j -> p t c j", t=TPB),
                        in1=s2row[:].unsqueeze(2)
                            .broadcast_to([P, TPB, LOWC, P]),
                        op=mybir.AluOpType.mult,
                    )
                    nc.vector.tensor_tensor(
                        out=tmp[:, L:CPB, :].rearrange("p (t c) j -> p t c j", t=TPB),
                        in0=o01[:, L:CPB, :].rearrange("p (t c) j -> p t c j", t=TPB),
                        in1=s2row[:].unsqueeze(2)
                            .broadcast_to([P, TPB, HIGHC, P]),
                        op=mybir.AluOpType.mult,
                    )
                    s2pe = sp.tile([P, CPB], F32, tag="s2pe")
                    nc.vector.tensor_reduce(
                        out=s2pe[:], in_=tmp[:],
                        axis=mybir.AxisListType.X,
                        op=mybir.AluOpType.add,
                    )

                    # per-edge math on [P, CPB] tiles
                    s1pe = sp.tile([P, CPB], F32, tag="s1pe")
                    nc.vector.tensor_copy(
                        out=s1pe[:].unsqueeze(2),
                        in_=rows[:, :, C_S1:C_S1 + 1],
                    )
                    # probs = B * prod_b (1 + bit_b*((mu/64)^(2^b)-1)) * e^-mu
                    bits3 = bits_sb[:].rearrange("p (s n) -> p s n", s=6)
                    pr = sp.tile([P, CPB], F32, tag="pr")
                    nc.vector.tensor_scalar(
                        out=pr[:], in0=bits3[:, 0, b * CPB:(b + 1) * CPB],
                        scalar1=mpm1[:, 0:1], scalar2=1.0,
                        op0=mybir.AluOpType.mult, op1=mybir.AluOpType.add,
                    )
                    fb = sp.tile([P, CPB], F32, tag="fb")
                    for b_ in range(1, 6):
                        nc.vector.tensor_scalar(
                            out=fb[:], in0=bits3[:, b_, b * CPB:(b + 1) * CPB],
                            scalar1=mpm1[:, b_:b_ + 1], scalar2=1.0,
                            op0=mybir.AluOpType.mult, op1=mybir.AluOpType.add,
                        )
                        nc.vector.tensor_tensor(out=pr[:], in0=pr[:], in1=fb[:],
                                                op=mybir.AluOpType.mult)
                    nc.vector.tensor_tensor(
                        out=pr[:], in0=pr[:],
                        in1=Bf_sb[:, b * CPB:(b + 1) * CPB],
                        op=mybir.AluOpType.mult,
                    )
                    probs = sp.tile([P, CPB], F32, tag="probs")
                    nc.vector.tensor_scalar(
                        out=probs[:], in0=pr[:],
                        scalar1=expnegmu[:, 0:1], scalar2=None,
                        op0=mybir.AluOpType.mult,
                    )
                    av = sp.tile([P, CPB], F32, tag="av")
                    nc.vector.tensor_tensor(out=av[:], in0=s1pe[:], in1=s2pe[:],
                                            op=mybir.AluOpType.add)
                    pa = sp.tile([P, CPB], F32, tag="pa")
                    nc.vector.tensor_tensor(out=pa[:], in0=probs[:], in1=av[:],
                                            op=mybir.AluOpType.mult)
                    pa2 = sp.tile([P, CPB], F32, tag="pa2")
                    nc.vector.tensor_scalar_mul(out=pa2[:], in0=pa[:],
                                                scalar1=NEG_SLOPE)
                    ev = sp.tile([P, CPB], F32, tag="ev")
                    nc.vector.tensor_tensor(out=ev[:], in0=pa[:], in1=pa2[:],
                                            op=mybir.AluOpType.max)
                    exb = sp.tile([P, CPB], F16, tag="exb")
                    nc.scalar.activation(
                        out=exb[:], in_=ev[:],
                        func=mybir.ActivationFunctionType.Exp,
                    )
                    exd = sp.tile([P, CPB], F32, tag="exd")
                    nc.vector.tensor_copy(out=exd[:], in_=exb[:])
                    nc.sync.dma_start(out=dbg_ex[:, b * CPB:(b + 1) * CPB], in_=exd[:])
                    nc.sync.dma_start(out=dbg_s2[:, b * CPB:(b + 1) * CPB], in_=s2pe[:])
                    nc.sync.dma_start(out=dbg_s1[:, b * CPB:(b + 1) * CPB], in_=s1pe[:])
                    # ex-weighted one-hot
                    ow = op_.tile([P, CPB, P], F16, tag="scratch", name="ow")
                    nc.vector.tensor_tensor(
                        out=ow[:], in0=o01[:],
                        in1=exb[:].unsqueeze(2).broadcast_to([P, CPB, P]),
                        op=mybir.AluOpType.mult,
                    )

                    if STAGE < 5:
                        continue
                    # matmuls: per dst tile accumulate [S | denom]
                    for t in range(TPB):
                        spsum = sps.tile([P, C_S1], F32, space="PSUM", tag="S")
                        for j in range(LOWC):
                            cb = t * LOWC + j
                            nc.tensor.matmul(
                                out=spsum[:],
                                lhsT=ow[:, cb, :],
                                rhs=rows[:, cb, 0:C_S1],
                                start=(j == 0), stop=False,
                            )
                        for j in range(HIGHC):
                            cb = TPB * LOWC + t * HIGHC + j
                            nc.tensor.matmul(
                                out=spsum[:],
                                lhsT=ow[:, cb, :],
                                rhs=rows[:, cb, 0:C_S1],
                                start=False, stop=(j == HIGHC - 1),
                            )
                        dn = sp.tile([P, 1], F32, tag="dn")
                        nc.vector.tensor_scalar_add(
                            out=dn[:], in0=spsum[:, C_ONE:C_ONE + 1],
                            scalar1=1e-30,
                        )
                        rec = sp.tile([P, 1], F32, tag="rec")
                        nc.vector.reciprocal(out=rec[:], in_=dn[:])
                        r0d = (b * TPB + t) * P
                        nc.sync.dma_start(out=dbg_dn[r0d:r0d + P, :], in_=dn[:])
                        ho = outp.tile([P, OUT_DIM], F32, tag="ho")
                        nc.vector.tensor_scalar(
                            out=ho[:], in0=spsum[:, 0:OUT_DIM],
                            scalar1=rec[:, 0:1], scalar2=None,
                            op0=mybir.AluOpType.mult,
                        )
                        r0 = (b * TPB + t) * P
                        nc.sync.dma_start(out=hout[r0:r0 + P, :], in_=ho[:])

    nc.compile()
    return nc


def _host_prep(h, fc_w, attn_w, src, dst, dist):
    E = src.shape[0]

    # folded attention weights
    v1 = fc_w.T.astype(np.float64) @ attn_w[0, :OUT_DIM].astype(np.float64)
    v2 = fc_w.T.astype(np.float64) @ attn_w[0, OUT_DIM:].astype(np.float64)
    W = np.zeros((IN_DIM, TROW), np.float32)
    W[:, :OUT_DIM] = fc_w.T
    W[:, C_S1] = v1.astype(np.float32)
    W[:, C_S2] = v2.astype(np.float32)
    W16 = W.astype(np.float16)

    h_pad = np.zeros((NC * RPC, IN_DIM), np.float32)
    h_pad[:N_NODES] = h
    hTs = [np.ascontiguousarray(h_pad[c * RPC:(c + 1) * RPC].T) for c in range(NC)]

    distp = np.zeros(P * DIST_COLS, np.int32)
    distp[:E] = dist
    distp = distp.reshape(P, DIST_COLS)

    kmax = int(dist.max()) // 2
    # B[k] = 64^k / k!  (exact host constants, in-range for k <= 60)
    Btab = np.array([math.exp(k * math.log(64.0) - math.lgamma(k + 1.0))
                     for k in range(kmax + 1)], np.float64)
    k_e = (dist // 2).astype(np.int64)

    # ---- edge bucketing ----
    tiles_e = dst.astype(np.int64) // P            # [0, 391]
    trow = _table_row(src.astype(np.int64))        # table row of src
    is_high = (trow >= LOWSPLIT).astype(np.int64)

    lowcnt = np.bincount(tiles_e * 2 + is_high, minlength=2 * (NC * TPC))
    LOWC = max(1, (int(lowcnt[0::2].max()) + P - 1) // P)
    HIGHC = max(1, (int(lowcnt[1::2].max()) + P - 1) // P)

    CPT = LOWC + HIGHC
    CPB = TPB * CPT
    NCHUNK = TCORE * CPT
    SEGL = TPB * LOWC * P // 16
    SEGH = TPB * HIGHC * P // 16

    key = tiles_e * 2 + is_high
    order = np.argsort(key, kind="stable")
    sk = key[order]
    gc = np.bincount(sk, minlength=2 * NC * TPC)
    group_start = np.zeros(2 * NC * TPC, np.int64)
    group_start[1:] = np.cumsum(gc)[:-1]
    pos = np.arange(E, dtype=np.int64) - group_start[sk]

    e_tile = tiles_e[order]
    e_high = is_high[order]
    e_core = e_tile // TPC
    tl = e_tile - e_core * TPC                     # tile index within core
    bat = tl // TPB
    tib = tl % TPB
    cb = np.where(
        e_high == 0,
        bat * CPB + tib * LOWC + pos // P,
        bat * CPB + TPB * LOWC + tib * HIGHC + pos // P,
    )
    pp_ = pos % P

    src_slot = np.zeros((NC, P, NCHUNK), np.int64)        # pad -> row 0
    dl_slot = np.full((NC, P, NCHUNK), -1.0, np.float16)
    bits_slot = np.zeros((NC, P, 6, NCHUNK), np.float16)
    Bf_slot = np.ones((NC, P, NCHUNK), np.float32)        # pad: k=0 -> B=1

    flat = (e_core * P + pp_) * NCHUNK + cb
    src_slot.reshape(-1)[flat] = trow[order]
    dl_slot.reshape(-1)[flat] = (dst[order].astype(np.int64) - e_tile * P
                                 ).astype(np.float16)
    ko = k_e[order]
    for b_ in range(6):
        flat_b = ((e_core * P + pp_) * 6 + b_) * NCHUNK + cb
        bits_slot.reshape(-1)[flat_b] = ((ko >> b_) & 1).astype(np.float16)
    Bf_slot.reshape(-1)[flat] = Btab[ko].astype(np.float32)

    def wrap_idx(vals):
        A = vals.reshape(-1, 16).T.astype(np.int16)       # [16, n/16]
        return np.tile(A, (8, 1))                         # [128, n/16]

    iota_arr = np.tile(np.arange(P, dtype=np.float16)[None, :],
                       (P, CPB)).reshape(P, CPB * P)

    in_maps = []
    for c in range(NC):
        gil_parts = []
        gih_parts = []
        for b in range(NB):
            lo = src_slot[c][:, b * CPB: b * CPB + TPB * LOWC]
            gil_parts.append(wrap_idx(lo.T.ravel()))
            hi = src_slot[c][:, b * CPB + TPB * LOWC: (b + 1) * CPB]
            hv = hi.T.ravel() - LOWSPLIT
            hv[hv < 0] = 0                                # pads (row 0)
            gih_parts.append(wrap_idx(hv))
        in_maps.append({
            "hT": hTs[c],
            "W": W16,
            "distp": distp,
            "gil": np.ascontiguousarray(np.concatenate(gil_parts, axis=1)),
            "gih": np.ascontiguousarray(np.concatenate(gih_parts, axis=1)),
            "dl": dl_slot[c],
            "bits": bits_slot[c].reshape(P, 6 * NCHUNK),
            "Bf": Bf_slot[c],
            "iot": iota_arr,
        })
    return in_maps, (LOWC, HIGHC)


def kernel(h, fc_w, attn_w, src, dst, dist, _trace=False):
    h = np.asarray(h, np.float32)
    fc_w = np.asarray(fc_w, np.float32)
    attn_w = np.asarray(attn_w, np.float32)
    src = np.asarray(src, np.int32)
    dst = np.asarray(dst, np.int32)
    dist = np.asarray(dist, np.int32)

    in_maps, key = _host_prep(h, fc_w, attn_w, src, dst, dist)
    if key not in _prog_cache:
        _prog_cache[key] = _build_program(*key)
    nc = _prog_cache[key]
    res = run_bass_kernel_spmd(nc, in_maps, list(range(NC)), trace=_trace)
    out = np.zeros((NC * RPC, OUT_DIM), np.float32)
    for c in range(NC):
        ho = res.results[c]["hout"]
        out[c * RPC:(c + 1) * RPC] = ho[:TPC * P]
    if _trace:
        kernel.last_results = res
    return out[:N_NODES]
